# revision 6
# baseline (speedup 1.0000x reference)
"""Trainium2 Bass kernel for nn_AudioVisualModel loss.

Strategy (8 NeuronCores, data-parallel over audio batch x):
  - Each core owns 3 of the 24 audio batches (150 of 1200 audio tokens),
    and streams the FULL visual matrix (37632 x 768) once from HBM.
  - Per core: normalize audio rows on-chip, normalize visual rows on-chip
    (norms via fused DVE square+reduce), PE-transpose visual tiles to get
    the (d, j) layout, then a bf16 PE matmul produces all token sims for
    this core's audio shard.  Reductions (max over Nv, sum min(s,0)^2,
    temporal diff^2 sums) are fused directly on the PSUM tiles.
  - Device outputs per core: (24, 3) clip-sim partials (visual-batch x
    local-audio-batch) and (150, 2) per-audio-row partial sums for the two
    regularizer terms.  The final (24,24) InfoNCE + scalar assembly is done
    on host (576 elements).
"""

import math
import os
import sys

import numpy as np

sys.path.insert(0, "/opt/trn_rl_repo")

import concourse.bass as bass
import concourse.tile as tile
from concourse import bacc, mybir
from concourse import masks as bass_masks
from concourse.bass_utils import run_bass_kernel_spmd

# Problem shapes (hardcoded per contract).
B, Na, T, Nv, D = 24, 50, 8, 196, 768
NCORES = 8
XPC = B // NCORES              # audio batches per core = 3
AR = XPC * Na                  # audio rows per core = 150
J = B * T * Nv                 # visual rows total = 37632
JY = T * Nv                    # visual rows per y = 1568
NBLK = JY // 128               # full 128-row blocks per y = 12
JREM = JY - NBLK * 128         # remainder rows = 32
KC = D // 128                  # contraction chunks = 6
NCHUNK = 392                   # matmul N chunk = 2 * Nv
CPY = JY // NCHUNK             # chunks per y = 4
EPS = 1e-12

_CACHE = {}


def _build(temp: float, thr: float):
    """Build the Bass module (single SPMD program for all 8 cores)."""
    f32 = mybir.dt.float32
    bf16 = mybir.dt.bfloat16

    nc = bacc.Bacc(
        "TRN2",
        target_bir_lowering=False,
        debug=False,
        enable_asserts=False,
        num_devices=NCORES,
    )

    a_in = nc.dram_tensor("a", [AR, D], f32, kind="ExternalInput").ap()
    v_in = nc.dram_tensor("v", [J, D], f32, kind="ExternalInput").ap()
    ind_in = nc.dram_tensor("ind", [AR, XPC], f32, kind="ExternalInput").ap()
    clip_out = nc.dram_tensor("clip", [B, XPC], f32, kind="ExternalOutput").ap()
    acc_out = nc.dram_tensor("acc", [AR, 2], f32, kind="ExternalOutput").ap()

    MT = [(0, 128), (1, AR - 128)]  # audio partition tiles

    with tile.TileContext(nc) as tc:
        from contextlib import ExitStack

        ctx = ExitStack()
        with ctx:
            singles = ctx.enter_context(tc.tile_pool(name="singles", bufs=1))
            vpool = ctx.enter_context(tc.tile_pool(name="vload", bufs=2))
            vtpool = ctx.enter_context(tc.tile_pool(name="vt", bufs=2))
            scrpool = ctx.enter_context(tc.tile_pool(name="scr", bufs=2))
            smpool = ctx.enter_context(tc.tile_pool(name="sm", bufs=2))
            tiny = ctx.enter_context(tc.tile_pool(name="tiny", bufs=3))
            mmpool = ctx.enter_context(
                tc.tile_pool(name="mm", bufs=5, space="PSUM")
            )
            tppool = ctx.enter_context(
                tc.tile_pool(name="tp", bufs=2, space="PSUM")
            )
            clpool = ctx.enter_context(
                tc.tile_pool(name="cl", bufs=1, space="PSUM")
            )

            ident = singles.tile([128, 128], bf16)
            bass_masks.make_identity(nc, ident[:])

            # ---------------- audio prep ----------------
            # aT[k] holds the k-th 128-row d-chunk of normalized-audio^T.
            aT = singles.tile([128, KC, AR], bf16)
            for mi, M in MT:
                at = tiny.tile([128, D], f32, tag="aload")
                nc.sync.dma_start(out=at[:M], in_=a_in[mi * 128 : mi * 128 + M, :])
                scr = tiny.tile([128, D], f32, tag="ascr")
                n2 = tiny.tile([128, 1], f32, tag="an2")
                nc.vector.affine_mul_reduce(
                    out=scr[:M],
                    accum_out=n2[:M],
                    in0=at[:M],
                    in1=at[:M],
                    scale=1.0,
                    bias=0.0,
                )
                nrm = tiny.tile([128, 1], f32, tag="anrm")
                nc.scalar.activation(
                    nrm[:M], n2[:M], mybir.ActivationFunctionType.Sqrt
                )
                nc.vector.tensor_scalar_max(nrm[:M], nrm[:M], EPS)
                rn = tiny.tile([128, 1], f32, tag="arn")
                nc.vector.reciprocal(rn[:M], nrm[:M])
                ab = tiny.tile([128, D], bf16, tag="ab")
                nc.vector.tensor_scalar_mul(ab[:M], at[:M], rn[:M])
                for k in range(KC):
                    pt = tppool.tile([128, 128], bf16)
                    nc.tensor.transpose(
                        pt[:, :M],
                        ab[:M, k * 128 : (k + 1) * 128],
                        ident[:M, :M],
                    )
                    eng = nc.vector if k % 2 == 0 else nc.scalar
                    if eng is nc.vector:
                        nc.vector.tensor_copy(
                            aT[:, k, mi * 128 : mi * 128 + M], pt[:, :M]
                        )
                    else:
                        nc.scalar.copy(
                            aT[:, k, mi * 128 : mi * 128 + M], pt[:, :M]
                        )

            # Per-audio-row accumulator columns (one col per y).
            maxv = [singles.tile([128, B * T], f32, tag=f"maxv{mi}", name=f"maxv{mi}") for mi, _ in MT]
            nncol = [singles.tile([128, B], f32, tag=f"nn{mi}", name=f"nn{mi}") for mi, _ in MT]
            tdcol = [singles.tile([128, B], f32, tag=f"td{mi}", name=f"td{mi}") for mi, _ in MT]

            inv_t2 = 1.0 / (temp * temp)

            # ---------------- visual sweep ----------------
            for y in range(B):
                vb = vpool.tile([128, NBLK + 1, D], bf16, tag="vb")
                src = v_in[y * JY : y * JY + NBLK * 128, :].rearrange(
                    "(b p) d -> p b d", p=128
                )
                nc.gpsimd.dma_start(out=vb[:, :NBLK, :], in_=src)
                nc.gpsimd.dma_start(
                    out=vb[:JREM, NBLK, :],
                    in_=v_in[y * JY + NBLK * 128 : (y + 1) * JY, :],
                )

                # row norms -> rn (128, NBLK+1)
                n2c = tiny.tile([128, NBLK + 1], f32, tag="n2c")
                for b in range(NBLK + 1):
                    P = 128 if b < NBLK else JREM
                    scrv = scrpool.tile([128, D], bf16, tag="scrv")
                    nc.vector.affine_mul_reduce(
                        out=scrv[:P],
                        accum_out=n2c[:P, b : b + 1],
                        in0=vb[:P, b, :],
                        in1=vb[:P, b, :],
                        scale=1.0,
                        bias=0.0,
                    )
                nrmv = tiny.tile([128, NBLK + 1], f32, tag="nrmv")
                # sqrt(n2 * temp^2) = ||v|| * temp
                nc.scalar.activation(
                    nrmv[:],
                    n2c[:],
                    mybir.ActivationFunctionType.Sqrt,
                    scale=float(temp * temp),
                )
                nc.vector.tensor_scalar_max(nrmv[:], nrmv[:], EPS)
                rnv = tiny.tile([128, NBLK + 1], f32, tag="rnv")
                nc.vector.reciprocal(rnv[:], nrmv[:])

                # normalize rows in place (bf16, 4x mode)
                for b in range(NBLK + 1):
                    P = 128 if b < NBLK else JREM
                    nc.vector.tensor_scalar_mul(
                        vb[:P, b, :], vb[:P, b, :], rnv[:P, b : b + 1]
                    )

                # transpose to vt (128, KC, JY)
                vt = vtpool.tile([128, KC, JY], bf16, tag="vt")
                ei = 0
                for b in range(NBLK + 1):
                    P = 128 if b < NBLK else JREM
                    for k in range(KC):
                        pt = tppool.tile([128, 128], bf16)
                        nc.tensor.transpose(
                            pt[:, :P],
                            vb[:P, b, k * 128 : (k + 1) * 128],
                            ident[:P, :P],
                        )
                        if ei % 2 == 0:
                            nc.vector.tensor_copy(
                                vt[:, k, b * 128 : b * 128 + P], pt[:, :P]
                            )
                        else:
                            nc.scalar.copy(
                                vt[:, k, b * 128 : b * 128 + P], pt[:, :P]
                            )
                        ei += 1

                # main matmul + fused reductions
                m_y = [
                    smpool.tile([128, JY], bf16, tag=f"m{mi}", name=f"m{mi}") for mi, _ in MT
                ]
                s_sb = [
                    smpool.tile([128, JY], bf16, tag=f"s{mi}", name=f"s{mi}") for mi, _ in MT
                ]
                dif_y = [
                    smpool.tile([128, (T - 1) * Nv], bf16, tag=f"dif{mi}", name=f"dif{mi}")
                    for mi, _ in MT
                ]
                for c in range(CPY):
                    for mi, M in MT:
                        ps = mmpool.tile([128, NCHUNK], f32, tag="ps")
                        for k in range(KC):
                            nc.tensor.matmul(
                                ps[:M],
                                lhsT=aT[:, k, mi * 128 : mi * 128 + M],
                                rhs=vt[:, k, c * NCHUNK : (c + 1) * NCHUNK],
                                start=(k == 0),
                                stop=(k == KC - 1),
                            )
                        # max over Nv for the two t-groups in this chunk
                        nc.vector.reduce_max(
                            maxv[mi][:M, y * T + 2 * c : y * T + 2 * c + 2],
                            ps[:M].rearrange("p (t v) -> p t v", v=Nv),
                            axis=mybir.AxisListType.X,
                        )
                        # clip(s, -20, 0) staged for nonneg reg
                        nc.vector.tensor_scalar(
                            out=m_y[mi][:M, c * NCHUNK : (c + 1) * NCHUNK],
                            in0=ps[:M],
                            scalar1=0.0,
                            scalar2=-20.0,
                            op0=mybir.AluOpType.min,
                            op1=mybir.AluOpType.max,
                        )
                        # stage sims to SBUF (bf16) for the temporal diffs
                        nc.scalar.copy(
                            s_sb[mi][:M, c * NCHUNK : (c + 1) * NCHUNK],
                            ps[:M],
                        )
                for mi, M in MT:
                    # temporal diffs from the staged SBUF sims
                    sv = s_sb[mi][:M].rearrange("p (t v) -> p t v", v=Nv)
                    dv = dif_y[mi][:M].rearrange("p (t v) -> p t v", v=Nv)
                    for t in range(T - 1):
                        nc.vector.tensor_tensor(
                            out=dv[:, t, :],
                            in0=sv[:, t + 1, :],
                            in1=sv[:, t, :],
                            op=mybir.AluOpType.subtract,
                        )
                for mi, M in MT:
                    scrm = scrpool.tile([128, JY], bf16, tag="scrm")
                    nc.scalar.activation(
                        scrm[:M],
                        m_y[mi][:M],
                        mybir.ActivationFunctionType.Square,
                        accum_out=nncol[mi][:M, y : y + 1],
                    )
                    scrd = scrpool.tile([128, (T - 1) * Nv], bf16, tag="scrd")
                    nc.scalar.activation(
                        scrd[:M],
                        dif_y[mi][:M],
                        mybir.ActivationFunctionType.Square,
                        accum_out=tdcol[mi][:M, y : y + 1],
                    )

            # ---------------- epilogue ----------------
            psc = clpool.tile([B, XPC], f32)
            for mi, M in MT:
                mask = tiny.tile([128, B * T], f32, tag=f"mask{mi}")
                nc.vector.tensor_scalar(
                    out=mask[:M],
                    in0=maxv[mi][:M],
                    scalar1=thr,
                    scalar2=None,
                    op0=mybir.AluOpType.is_ge,
                )
                msked = tiny.tile([128, B * T], f32, tag=f"msk{mi}")
                nc.vector.tensor_tensor(
                    out=msked[:M],
                    in0=maxv[mi][:M],
                    in1=mask[:M],
                    op=mybir.AluOpType.mult,
                )
                counts = tiny.tile([128, B], f32, tag=f"cnt{mi}")
                nc.vector.reduce_sum(
                    counts[:M],
                    mask[:M].rearrange("p (y t) -> p y t", t=T),
                    axis=mybir.AxisListType.X,
                )
                toksum = tiny.tile([128, B], f32, tag=f"tks{mi}")
                nc.vector.reduce_sum(
                    toksum[:M],
                    msked[:M].rearrange("p (y t) -> p y t", t=T),
                    axis=mybir.AxisListType.X,
                )
                nc.vector.tensor_scalar_max(counts[:M], counts[:M], 1.0)
                rcc = tiny.tile([128, B], f32, tag=f"rcc{mi}")
                nc.vector.reciprocal(rcc[:M], counts[:M])
                tok = tiny.tile([128, B], f32, tag=f"tok{mi}")
                nc.vector.tensor_tensor(
                    out=tok[:M],
                    in0=toksum[:M],
                    in1=rcc[:M],
                    op=mybir.AluOpType.mult,
                )
                # mean over audio tokens within each local x: ones-matmul
                ind = tiny.tile([128, XPC], f32, tag=f"ind{mi}", name=f"ind{mi}")
                nc.sync.dma_start(
                    out=ind[:M], in_=ind_in[mi * 128 : mi * 128 + M, :]
                )
                nc.tensor.matmul(
                    psc[:, :],
                    lhsT=tok[:M],
                    rhs=ind[:M],
                    start=(mi == 0),
                    stop=(mi == 1),
                )
                # regularizer partials -> acc_out rows
                accs = tiny.tile([128, 2], f32, tag=f"accs{mi}")
                nc.vector.reduce_sum(
                    accs[:M, 0:1], nncol[mi][:M], axis=mybir.AxisListType.X
                )
                nc.vector.reduce_sum(
                    accs[:M, 1:2], tdcol[mi][:M], axis=mybir.AxisListType.X
                )
                nc.sync.dma_start(
                    out=acc_out[mi * 128 : mi * 128 + M, :], in_=accs[:M]
                )
            cls = tiny.tile([B, XPC], f32, tag="cls")
            nc.vector.tensor_copy(cls[:], psc[:])
            nc.sync.dma_start(out=clip_out[:, :], in_=cls[:])

    nc.compile()
    return nc


def kernel(audio_feats, visual_feats, temperature, threshold):
    temp = float(np.asarray(temperature))
    thr_in = float(np.asarray(threshold))
    thr = 1.0 / (1.0 + math.exp(-thr_in))  # sigmoid

    key = (temp, thr_in)
    if key not in _CACHE:
        _CACHE[key] = _build(temp, thr)
    nc = _CACHE[key]

    a = np.ascontiguousarray(
        np.asarray(audio_feats, dtype=np.float32).reshape(B * Na, D)
    )
    v = np.ascontiguousarray(
        np.asarray(visual_feats, dtype=np.float32).reshape(J, D)
    )

    ind = np.zeros((AR, XPC), dtype=np.float32)
    for g in range(XPC):
        ind[g * Na : (g + 1) * Na, g] = 1.0 / Na

    in_maps = []
    for c in range(NCORES):
        in_maps.append({"a": a[c * AR : (c + 1) * AR], "v": v, "ind": ind})

    res = run_bass_kernel_spmd(nc, in_maps, core_ids=list(range(NCORES)))
    outs = res.results

    # host assembly (576-element InfoNCE + scalar reg terms)
    clip = np.zeros((B, B), dtype=np.float64)
    s_nonneg = 0.0
    s_tdiff = 0.0
    for c in range(NCORES):
        co = outs[c]["clip"].astype(np.float64)  # (B=y, XPC=g)
        for g in range(XPC):
            clip[c * XPC + g, :] = co[:, g]
        acc = outs[c]["acc"].astype(np.float64)  # (AR, 2)
        s_nonneg += acc[:, 0].sum()
        s_tdiff += acc[:, 1].sum()

    def logsumexp(m, axis):
        mx = m.max(axis=axis, keepdims=True)
        return mx + np.log(np.exp(m - mx).sum(axis=axis, keepdims=True))

    diag = np.arange(B)
    lsm1 = clip - logsumexp(clip, 1)
    lsm0 = clip - logsumexp(clip, 0)
    contrastive = -(lsm1[diag, diag] + lsm0[diag, diag]).mean() / 2.0

    l_nonneg = s_nonneg / (B * B * Na * T * Nv)
    l_temporal = s_tdiff / (B * B * Na * (T - 1) * Nv)
    log_t = math.log(temp)
    temp_low = max(math.log(2.3) - log_t, 0.0) ** 3
    temp_high = max(log_t - math.log(4.0), 0.0) ** 3
    reg = 0.15 * l_nonneg + 8.0 * (temp_low + temp_high) + 0.01 * l_temporal

    return np.float32(contrastive + reg)
